# revision 16
# baseline (speedup 1.0000x reference)
"""Anchor3DHead head (three 1x1 convs) as one fused channel-contraction
matmul, sharded over 8 TRN2 NeuronCores.

Math: for x [B, C, H, W] and W_cat = [w_cls | w_reg | w_dir] ([C, 20]),
    out[b, o, h, w] = sum_c x[b, c, h, w] * W_cat[c, o] + b_cat[o]
Each core handles one (batch, H-half) shard: rhs = x-shard [C, 26784]
streamed through the PE with W_cat chunks stationary, accumulated over
3 K-chunks of 128 channels into PSUM, bias added on the vector engine.
"""

import numpy as np

import concourse.bacc as bacc
import concourse.mybir as mybir
from concourse.bass_utils import run_bass_kernel_spmd
from concourse.tile import TileContext

B, C, H, W = 4, 384, 248, 216
O_CLS, O_REG, O_DIR = 2, 14, 4
O = O_CLS + O_REG + O_DIR      # 20
N_CORES = 8
H_SH = H // 2                  # 124 H-rows per core (4 batches x 2 H-halves)
N_SH = H_SH * W                # 26784 spatial positions per core
K_CHUNKS = C // 128            # 3

N_CHUNK = 1488                 # spatial cols per x DMA (2.29 MB per [128, 3, N_CHUNK] tile)
N_SUB = 496                    # cols per matmul (<=512 fp32 / one PSUM bank)

MM_DTYPE = mybir.dt.float32r   # fp32 bits, PE multiplies at reduced precision, 4x rate


def build_nc(n_sh=N_SH, n_chunk=N_CHUNK, n_sub=N_SUB, mm_dtype=MM_DTYPE,
             x_bufs=8, o_bufs=3, ps_bufs=8):
    assert n_sh % n_chunk == 0 and n_chunk % n_sub == 0
    nc = bacc.Bacc(num_devices=N_CORES)
    xs = nc.dram_tensor("xs", [C, n_sh], mm_dtype, kind="ExternalInput")
    wcat = nc.dram_tensor("wcat", [C, O], mm_dtype, kind="ExternalInput")
    bcat = nc.dram_tensor("bcat", [O, 1], mybir.dt.float32, kind="ExternalInput")
    out = nc.dram_tensor("out", [O, n_sh], mybir.dt.float32, kind="ExternalOutput")

    with TileContext(nc) as tc:
        with tc.tile_pool(name="consts", bufs=1) as cpool, \
             tc.tile_pool(name="x", bufs=x_bufs) as xpool, \
             tc.tile_pool(name="o", bufs=o_bufs) as opool, \
             tc.tile_pool(name="ps", bufs=ps_bufs, space="PSUM") as ppool:
            # [C, n] DRAM views reshaped so all 3 K-chunks land in one DMA:
            # dest[p, k, ...] = src[k*128 + p, ...]
            xs_v = xs[:, :].rearrange("(k p) n -> p k n", k=K_CHUNKS)
            w_v = wcat[:, :].rearrange("(k p) o -> p k o", k=K_CHUNKS)

            # Ring split: x stream on the SP HWDGE ring (nc.sync); weights,
            # bias and output stores on the ACT ring (nc.scalar) so a
            # compute-gated out-DMA never stalls queued input DMAs (HWDGE
            # rings are FIFO per issuing engine).
            w_sb = cpool.tile([128, K_CHUNKS, O], mm_dtype)
            b_sb = cpool.tile([O, 1], mybir.dt.float32)

            for ci in range(n_sh // n_chunk):
                xt = xpool.tile([128, K_CHUNKS, n_chunk], mm_dtype, tag="xt")
                nc.sync.dma_start(
                    out=xt,
                    in_=xs_v[:, :, ci * n_chunk:(ci + 1) * n_chunk])
                if ci == 0:
                    nc.scalar.dma_start(out=w_sb, in_=w_v)
                    nc.scalar.dma_start(out=b_sb, in_=bcat[:, :])
                ot = opool.tile([O, n_chunk], mybir.dt.float32, tag="ot")
                for si in range(n_chunk // n_sub):
                    ps = ppool.tile([O, n_sub], mybir.dt.float32, tag="ps")
                    for k in range(K_CHUNKS):
                        nc.tensor.matmul(ps, lhsT=w_sb[:, k, :],
                                         rhs=xt[:, k, si * n_sub:(si + 1) * n_sub],
                                         start=(k == 0), stop=(k == K_CHUNKS - 1))
                    nc.vector.tensor_scalar_add(
                        ot[:, si * n_sub:(si + 1) * n_sub], ps, b_sb)
                nc.scalar.dma_start(out=out[:, ci * n_chunk:(ci + 1) * n_chunk],
                                    in_=ot)
    nc.compile()
    return nc


def shard_inputs(x, w_cls, b_cls, w_reg, b_reg, w_dir, b_dir):
    wcat = np.ascontiguousarray(
        np.concatenate([w_cls, w_reg, w_dir], axis=1), dtype=np.float32)
    bcat = np.ascontiguousarray(
        np.concatenate([b_cls, b_reg, b_dir]).reshape(O, 1), dtype=np.float32)
    in_maps = []
    for i in range(N_CORES):
        b, h0 = divmod(i, 2)
        xs = np.ascontiguousarray(
            x[b, :, h0 * H_SH:(h0 + 1) * H_SH, :], dtype=np.float32
        ).reshape(C, N_SH)
        in_maps.append({"xs": xs, "wcat": wcat, "bcat": bcat})
    return in_maps


def assemble_output(results):
    full = np.empty((B, O, H, W), dtype=np.float32)
    for i in range(N_CORES):
        b, h0 = divmod(i, 2)
        full[b, :, h0 * H_SH:(h0 + 1) * H_SH, :] = \
            results[i]["out"].reshape(O, H_SH, W)
    cls_score = np.ascontiguousarray(full[:, :O_CLS])
    bbox_pred = np.ascontiguousarray(full[:, O_CLS:O_CLS + O_REG])
    dir_cls = np.ascontiguousarray(full[:, O_CLS + O_REG:])
    return cls_score, bbox_pred, dir_cls


_NC_CACHE = {}


def run(x, w_cls, b_cls, w_reg, b_reg, w_dir, b_dir, build_kwargs=None,
        **spmd_kwargs):
    """Build (cached) + run on 8 cores; returns (outputs_tuple, BassKernelResults)."""
    key = tuple(sorted((build_kwargs or {}).items()))
    if key not in _NC_CACHE:
        _NC_CACHE[key] = build_nc(**(build_kwargs or {}))
    nc = _NC_CACHE[key]
    in_maps = shard_inputs(x, w_cls, b_cls, w_reg, b_reg, w_dir, b_dir)
    res = run_bass_kernel_spmd(nc, in_maps, list(range(N_CORES)), **spmd_kwargs)
    return assemble_output(res.results), res


def kernel(x, w_cls, b_cls, w_reg, b_reg, w_dir, b_dir):
    outs, _ = run(np.asarray(x), np.asarray(w_cls), np.asarray(b_cls),
                  np.asarray(w_reg), np.asarray(b_reg),
                  np.asarray(w_dir), np.asarray(b_dir))
    return outs


# revision 24
# speedup vs baseline: 1.0067x; 1.0067x over previous
"""Anchor3DHead head (three 1x1 convs) as one fused channel-contraction
matmul, sharded over 8 TRN2 NeuronCores.

Math: for x [B, C, H, W] and W_cat = [w_cls | w_reg | w_dir] ([C, 20]),
    out[b, o, h, w] = sum_c x[b, c, h, w] * W_cat[c, o] + b_cat[o]
Each core handles one (batch, H-half) shard: rhs = x-shard [C, 26784]
streamed through the PE with W_cat chunks stationary, accumulated over
3 K-chunks of 128 channels into PSUM, bias added on the vector engine.
"""

import numpy as np

import concourse.bacc as bacc
import concourse.mybir as mybir
from concourse.bass_utils import run_bass_kernel_spmd
from concourse.tile import TileContext

B, C, H, W = 4, 384, 248, 216
O_CLS, O_REG, O_DIR = 2, 14, 4
O = O_CLS + O_REG + O_DIR      # 20
N_CORES = 8
H_SH = H // 2                  # 124 H-rows per core (4 batches x 2 H-halves)
N_SH = H_SH * W                # 26784 spatial positions per core
K_CHUNKS = C // 128            # 3

N_CHUNK = 1488                 # spatial cols per x DMA (2.29 MB per [128, 3, N_CHUNK] tile)
N_SUB = 496                    # cols per matmul (<=512 fp32 / one PSUM bank)

MM_DTYPE = mybir.dt.float32r   # fp32 bits, PE multiplies at reduced precision, 4x rate


def build_nc(n_sh=N_SH, n_chunk=N_CHUNK, n_sub=N_SUB, mm_dtype=MM_DTYPE,
             x_bufs=8, o_bufs=3, ps_bufs=2):
    assert n_sh % n_chunk == 0 and n_chunk % n_sub == 0
    nc = bacc.Bacc(num_devices=N_CORES)
    xs = nc.dram_tensor("xs", [C, n_sh], mm_dtype, kind="ExternalInput")
    wcat = nc.dram_tensor("wcat", [C, O], mm_dtype, kind="ExternalInput")
    bcat = nc.dram_tensor("bcat", [O, 1], mybir.dt.float32, kind="ExternalInput")
    out = nc.dram_tensor("out", [O, n_sh], mybir.dt.float32, kind="ExternalOutput")

    with TileContext(nc) as tc:
        with tc.tile_pool(name="consts", bufs=1) as cpool, \
             tc.tile_pool(name="x", bufs=x_bufs) as xpool, \
             tc.tile_pool(name="o", bufs=o_bufs) as opool, \
             tc.tile_pool(name="ps", bufs=ps_bufs, space="PSUM") as ppool:
            # [C, n] DRAM views reshaped so all 3 K-chunks land in one DMA:
            # dest[p, k, ...] = src[k*128 + p, ...]
            xs_v = xs[:, :].rearrange("(k p) n -> p k n", k=K_CHUNKS)
            w_v = wcat[:, :].rearrange("(k p) o -> p k o", k=K_CHUNKS)

            # Ring split: x stream on the SP HWDGE ring (nc.sync); weights,
            # bias and output stores on the ACT ring (nc.scalar) so a
            # compute-gated out-DMA never stalls queued input DMAs (HWDGE
            # rings are FIFO per issuing engine).
            w_sb = cpool.tile([128, K_CHUNKS, O], mm_dtype)
            b_sb = cpool.tile([O, 1], mybir.dt.float32)

            for ci in range(n_sh // n_chunk):
                xt = xpool.tile([128, K_CHUNKS, n_chunk], mm_dtype, tag="xt")
                nc.sync.dma_start(
                    out=xt,
                    in_=xs_v[:, :, ci * n_chunk:(ci + 1) * n_chunk])
                if ci == 0:
                    nc.scalar.dma_start(out=w_sb, in_=w_v)
                    nc.scalar.dma_start(out=b_sb, in_=bcat[:, :])
                ot = opool.tile([O, n_chunk], mybir.dt.float32, tag="ot")
                n_groups = n_chunk // n_sub
                # One PSUM tile spanning n_groups banks (512-f32 stride so
                # each matmul window stays inside one bank); a single DVE
                # bias-add covers the whole chunk. Fewer instructions →
                # fewer multi-wait legalization sites → less event-sem
                # churn on the sequencers and at the exit sem-clear.
                ps = ppool.tile([O, n_groups, 512], mybir.dt.float32, tag="ps")
                for si in range(n_groups):
                    for k in range(K_CHUNKS):
                        nc.tensor.matmul(ps[:, si, :n_sub], lhsT=w_sb[:, k, :],
                                         rhs=xt[:, k, si * n_sub:(si + 1) * n_sub],
                                         start=(k == 0), stop=(k == K_CHUNKS - 1))
                ot_v = ot.rearrange("p (g s) -> p g s", s=n_sub)
                nc.vector.tensor_scalar_add(ot_v, ps[:, :, :n_sub], b_sb)
                nc.scalar.dma_start(out=out[:, ci * n_chunk:(ci + 1) * n_chunk],
                                    in_=ot)
    nc.compile()
    return nc


def build_raw(n_sh=N_SH, n_chunk=N_CHUNK, n_sub=N_SUB, mm_dtype=MM_DTYPE,
              x_bufs=8, o_bufs=3, ps_bufs=2):
    """Raw bacc pipeline (no TileContext): hand-rolled semaphore ring across
    Sync(x-DMA) -> Tensor(matmul) -> Vector(bias) -> Scalar(out-DMA), skipping
    Tile's ~3us entry barrier and ~7us exit sem-clear churn."""
    assert n_sh % n_chunk == 0 and n_chunk % n_sub == 0
    n_chunks = n_sh // n_chunk
    G = n_chunk // n_sub
    R, P, B = x_bufs, ps_bufs, o_bufs
    assert P * G * 512 <= 4096, "psum overflow"

    nc = bacc.Bacc(num_devices=N_CORES)
    xs = nc.dram_tensor("xs", [C, n_sh], mm_dtype, kind="ExternalInput")
    wcat = nc.dram_tensor("wcat", [C, O], mm_dtype, kind="ExternalInput")
    bcat = nc.dram_tensor("bcat", [O, 1], mybir.dt.float32, kind="ExternalInput")
    out = nc.dram_tensor("out", [O, n_sh], mybir.dt.float32, kind="ExternalOutput")
    xs_v = xs[:, :].rearrange("(k p) n -> p k n", k=K_CHUNKS)
    w_v = wcat[:, :].rearrange("(k p) o -> p k o", k=K_CHUNKS)

    from contextlib import ExitStack
    with ExitStack() as ctx:
        xt = ctx.enter_context(
            nc.sbuf_tensor([128, R, K_CHUNKS, n_chunk], mm_dtype))
        w_sb = ctx.enter_context(nc.sbuf_tensor([128, K_CHUNKS, O], mm_dtype))
        b_sb = ctx.enter_context(nc.sbuf_tensor([O, 1], mybir.dt.float32))
        ot = ctx.enter_context(
            nc.sbuf_tensor([O, B, n_chunk], mybir.dt.float32))
        ps = ctx.enter_context(nc.psum_tensor([O, P, G, 512], mybir.dt.float32))
        # Per-slot DMA sems: a cumulative counter would let chunk c+1's
        # completion release a waiter on chunk c (DMA completions are
        # unordered across the 16 SDMA engines).
        x_sems = [ctx.enter_context(nc.semaphore(f"x_sem{j}")) for j in range(R)]
        o_sems = [ctx.enter_context(nc.semaphore(f"o_sem{j}")) for j in range(B)]
        c_sem = ctx.enter_context(nc.semaphore("c_sem"))
        pe_sem = ctx.enter_context(nc.semaphore("pe_sem"))
        dve_sem = ctx.enter_context(nc.semaphore("dve_sem"))
        # Sync engine: the x-DMA stream.
        for ci in range(n_chunks):
            if ci >= R:
                nc.sync.wait_ge(pe_sem, ci - R + 1)
            nc.sync.dma_start(
                out=xt[:, ci % R],
                in_=xs_v[:, :, ci * n_chunk:(ci + 1) * n_chunk],
            ).then_inc(x_sems[ci % R], 16)

        # Scalar engine: const loads + output stores.
        nc.scalar.dma_start(out=w_sb[:, :, :], in_=w_v).then_inc(c_sem, 16)
        nc.scalar.dma_start(out=b_sb[:, :], in_=bcat[:, :]).then_inc(c_sem, 16)
        for ci in range(n_chunks):
            nc.scalar.wait_ge(dve_sem, ci + 1)
            nc.scalar.dma_start(
                out=out[:, ci * n_chunk:(ci + 1) * n_chunk],
                in_=ot[:, ci % B],
            ).then_inc(o_sems[ci % B], 16)
        for j in range(B):
            nc.scalar.wait_ge(o_sems[j], 16 * len(range(j, n_chunks, B)))
        for j in range(R):
            nc.scalar.wait_ge(x_sems[j], 16 * len(range(j, n_chunks, R)))
        nc.scalar.wait_ge(pe_sem, n_chunks)
        nc.scalar.wait_ge(c_sem, 32)

        # Tensor engine: matmuls.
        nc.tensor.wait_ge(c_sem, 32)
        for ci in range(n_chunks):
            nc.tensor.wait_ge(x_sems[ci % R], 16 * (ci // R + 1))
            if ci >= P:
                nc.tensor.wait_ge(dve_sem, ci - P + 1)
            mm = None
            for si in range(G):
                for k in range(K_CHUNKS):
                    mm = nc.tensor.matmul(
                        ps[:, ci % P, si, :n_sub], lhsT=w_sb[:, k, :],
                        rhs=xt[:, ci % R, k, si * n_sub:(si + 1) * n_sub],
                        start=(k == 0), stop=(k == K_CHUNKS - 1))
            mm.then_inc(pe_sem, 1)

        # Vector engine: bias-add PSUM -> SBUF.
        nc.vector.wait_ge(c_sem, 32)
        for ci in range(n_chunks):
            nc.vector.wait_ge(pe_sem, ci + 1)
            if ci >= B:
                nc.vector.wait_ge(o_sems[ci % B], 16 * ((ci - B) // B + 1))
            ot_v = ot[:, ci % B].rearrange("p (g s) -> p g s", s=n_sub)
            nc.vector.tensor_scalar_add(
                ot_v, ps[:, ci % P, :, :n_sub], b_sb[:, :],
            ).then_inc(dve_sem, 1)

        nc.all_engine_barrier()
        for s in x_sems + o_sems + [c_sem, pe_sem, dve_sem]:
            nc.scalar.sem_clear(s)

    nc.compile()
    return nc


def shard_inputs(x, w_cls, b_cls, w_reg, b_reg, w_dir, b_dir):
    wcat = np.ascontiguousarray(
        np.concatenate([w_cls, w_reg, w_dir], axis=1), dtype=np.float32)
    bcat = np.ascontiguousarray(
        np.concatenate([b_cls, b_reg, b_dir]).reshape(O, 1), dtype=np.float32)
    in_maps = []
    for i in range(N_CORES):
        b, h0 = divmod(i, 2)
        xs = np.ascontiguousarray(
            x[b, :, h0 * H_SH:(h0 + 1) * H_SH, :], dtype=np.float32
        ).reshape(C, N_SH)
        in_maps.append({"xs": xs, "wcat": wcat, "bcat": bcat})
    return in_maps


def assemble_output(results):
    full = np.empty((B, O, H, W), dtype=np.float32)
    for i in range(N_CORES):
        b, h0 = divmod(i, 2)
        full[b, :, h0 * H_SH:(h0 + 1) * H_SH, :] = \
            results[i]["out"].reshape(O, H_SH, W)
    cls_score = np.ascontiguousarray(full[:, :O_CLS])
    bbox_pred = np.ascontiguousarray(full[:, O_CLS:O_CLS + O_REG])
    dir_cls = np.ascontiguousarray(full[:, O_CLS + O_REG:])
    return cls_score, bbox_pred, dir_cls


_NC_CACHE = {}


def run(x, w_cls, b_cls, w_reg, b_reg, w_dir, b_dir, build_kwargs=None,
        **spmd_kwargs):
    """Build (cached) + run on 8 cores; returns (outputs_tuple, BassKernelResults)."""
    key = tuple(sorted((build_kwargs or {}).items()))
    if key not in _NC_CACHE:
        _NC_CACHE[key] = build_nc(**(build_kwargs or {}))
    nc = _NC_CACHE[key]
    in_maps = shard_inputs(x, w_cls, b_cls, w_reg, b_reg, w_dir, b_dir)
    res = run_bass_kernel_spmd(nc, in_maps, list(range(N_CORES)), **spmd_kwargs)
    return assemble_output(res.results), res


def kernel(x, w_cls, b_cls, w_reg, b_reg, w_dir, b_dir):
    outs, _ = run(np.asarray(x), np.asarray(w_cls), np.asarray(b_cls),
                  np.asarray(w_reg), np.asarray(b_reg),
                  np.asarray(w_dir), np.asarray(b_dir))
    return outs
